# revision 1
# baseline (speedup 1.0000x reference)
"""Trainium2 Bass kernel for nn_Recommender_2 (moe_routing).

Pipeline per core (8 NeuronCores, one TRN2 chip):
  Phase 1 (data-parallel over batch, 128 rows/core):
    - indirect-DMA gather of ratings embeddings (table pre-cast to bf16)
    - PE-transpose to time-major x^T tiles
    - 2-layer LSTM scan (T=50) in transposed [gate, batch] layout:
      weights stationary (lhsT), state h^T flows as rhs -> no per-step
      transposes. Gate order host-permuted to (i,f,o,g) so sigmoid/tanh
      batch into few ACT calls. Layer-0 input MMs pipelined one step ahead
      into dedicated PSUM tiles (off the critical chain, keeps PE warm);
      layer-1 recurrent MMs issued before its input MMs.
    - ratings MLP + user MLP (transposed layout) -> z^T [384,128] bf16
  Exchange: AllGather of z^T across the 8 cores -> Z^T [384,1024]
  Phase 2 (expert-parallel, 8 experts/core over full batch):
    - he = z @ W1_e accumulated in PSUM (bf16 weights)
    - the second GEMM (he @ w2_e, blocked by the relu) is eliminated; the
      reduction alternates engines per expert slot: even -> two ACT Relu
      accum_out calls over |w2|-folded sign-partitioned hidden units
      (s+ - s-); odd -> one DVE scalar_tensor_tensor (he max 0)*w2 with
      accum_out. ACT and DVE each drain half the PSUM tiles; PE streams.
Host reassembles [1024, 64] from per-core [1024, 8] outputs.
"""
import numpy as np
import ml_dtypes

import concourse.bacc as bacc
import concourse.bass as bass
import concourse.mybir as mybir
import concourse.tile as tile
from concourse.bass_utils import run_bass_kernel_spmd
from concourse.masks import make_identity

P = 128
NCORES = 8
B, T = 1024, 50
RV, RD, RH = 100000, 128, 256
R_OUT = 256
UV, UD, UDATA, U_OUT = 50000, 64, 32, 128
E, EIN, EH = 64, 384, 1536
EPC = E // NCORES
BL = B // NCORES
UIN = UD + UDATA
UH = 192
RHID = 512
NG = 8

F32 = mybir.dt.float32
BF16 = mybir.dt.bfloat16
I32 = mybir.dt.int32
AF = mybir.ActivationFunctionType
ALU = mybir.AluOpType

_cache = {}


def _prep(inputs):
    f = lambda k: np.asarray(inputs[k], dtype=np.float32)
    bf = lambda a: np.ascontiguousarray(a, dtype=ml_dtypes.bfloat16)
    f32c = lambda a: np.ascontiguousarray(a, dtype=np.float32)

    # gate permutation i,f,g,o -> i,g,f,o  (i,g in PSUM bank0: the c-update
    # chain starts after only half the recurrent matmuls)
    perm = np.r_[0:256, 512:768, 256:512, 768:1024]

    shared = {}
    lstm_bias = False
    for l in range(2):
        wih = f(f"lstm_W_ih_{l}")[perm]
        whh = f(f"lstm_W_hh_{l}")[perm]
        bias = (f(f"lstm_b_ih_{l}") + f(f"lstm_b_hh_{l}"))[perm]
        shared[f"wih{l}"] = bf(wih.T)            # [in, 1024]
        shared[f"whh{l}"] = bf(whh.T)            # [256, 1024]
        shared[f"lb{l}"] = f32c(bias.reshape(NG, P).T)
        lstm_bias = lstm_bias or bool(np.any(bias))

    shared["rw1"] = bf(f("r_W1"))
    shared["rw2"] = bf(f("r_W2"))
    shared["uw1"] = bf(f("u_W1"))
    shared["uw2"] = bf(f("u_W2"))
    mlpb = np.zeros((P, 9), np.float32)
    mlpb[:, 0:4] = f("r_b1").reshape(4, P).T
    mlpb[:, 4:6] = f("r_b2").reshape(2, P).T
    ub1 = f("u_b1")
    mlpb[:, 6] = ub1[0:128]
    mlpb[0:64, 7] = ub1[128:192]
    mlpb[:, 8] = f("u_b2")
    shared["mlpb"] = mlpb

    shared["remb"] = bf(f("ratings_emb"))        # bf16 table
    shared["uemb"] = f32c(f("user_emb"))

    # Experts, hybrid reduction so consumers alternate between ACT and DVE:
    #  even local slot -> |w2| folded into W1, hidden units permuted into
    #    [positives | pad | negatives | pad] (core-uniform boundary Bpos);
    #    reduction = two ACT Relu accum_out calls (s+ - s-).
    #  odd local slot -> native W1; reduction = one DVE scalar_tensor_tensor
    #    (he max 0) * w2 with accum_out.
    w1 = f("exp_W1")                             # [64, 384, 1536]
    w2 = f("exp_W2").reshape(E, EH)              # [64, 1536]
    b1 = f("exp_b1")                             # [64, 1536]
    b2 = f("exp_b2").reshape(E)
    exp_b1_nz = bool(np.any(b1))
    pos_masks = w2 >= 0
    slot = np.arange(E) % EPC
    p_cnt = pos_masks.sum(1)
    n_cnt = EH - p_cnt
    even_mask = slot % 2 == 0
    Bpos = int(np.ceil(p_cnt[even_mask].max() / 128.0) * 128)
    Bneg = int(np.ceil(n_cnt[even_mask].max() / 128.0) * 128)
    H2 = Bpos + Bneg
    w1f = np.zeros((E, EIN, H2), np.float32)
    b1f = np.zeros((E, H2), np.float32)
    for e in range(E):
        if slot[e] % 2 == 0:
            pos = np.nonzero(pos_masks[e])[0]
            neg = np.nonzero(~pos_masks[e])[0]
            scaled = w1[e] * np.abs(w2[e])[None, :]
            w1f[e, :, 0:len(pos)] = scaled[:, pos]
            w1f[e, :, Bpos:Bpos + len(neg)] = scaled[:, neg]
            bsc = b1[e] * np.abs(w2[e])
            b1f[e, 0:len(pos)] = bsc[pos]
            b1f[e, Bpos:Bpos + len(neg)] = bsc[neg]
        else:
            w1f[e, :, 0:EH] = w1[e]
            b1f[e, 0:EH] = b1[e]

    ridx = np.asarray(inputs["ratings_tensor"]).astype(np.int32)
    uids = np.asarray(inputs["user_ids"]).astype(np.int32)
    udata = f("user_data")

    in_maps = []
    for c in range(NCORES):
        m = dict(shared)
        sl = slice(c * BL, (c + 1) * BL)
        m["ridx"] = np.ascontiguousarray(ridx[sl])
        m["uid"] = np.ascontiguousarray(uids[sl].reshape(BL, 1))
        m["udata"] = f32c(udata[sl])
        es = slice(c * EPC, (c + 1) * EPC)
        m["expw"] = bf(w1f[es])                  # [8, 384, H2]
        m["w2r"] = f32c(np.broadcast_to(
            w2[es][1::2][:, None, :], (EPC // 2, P, EH)))
        m["b1p"] = bf(b1f[es])                   # [8, H2]
        b2bc = np.zeros((P, E), np.float32)
        for cc in range(NCORES):
            for e in range(EPC):
                b2bc[:, cc * EPC + e] = b2[c * EPC + e]
        m["b2bc"] = b2bc
        in_maps.append(m)

    bp = dict(lstm_bias=lstm_bias, exp_b1=exp_b1_nz, exp_b2=bool(np.any(b2)),
              H2=H2, Bpos=Bpos)
    return in_maps, bp


def _build(bp, sim_single=False):
    H2, Bpos = bp["H2"], bp["Bpos"]
    NH_full = [(s, min(s + 512, H2)) for s in range(0, H2, 512)]
    NH_nat = [(s, s + 512) for s in range(0, EH, 512)]

    nc = bacc.Bacc("TRN2", target_bir_lowering=False)
    d_ridx = nc.dram_tensor("ridx", [BL, T], I32, kind="ExternalInput")
    d_uid = nc.dram_tensor("uid", [BL, 1], I32, kind="ExternalInput")
    d_udata = nc.dram_tensor("udata", [BL, UDATA], F32, kind="ExternalInput")
    d_remb = nc.dram_tensor("remb", [RV, RD], BF16, kind="ExternalInput")
    d_uemb = nc.dram_tensor("uemb", [UV, UD], F32, kind="ExternalInput")
    d_wih = [nc.dram_tensor(f"wih{l}", [RD if l == 0 else RH, 4 * RH], BF16,
                            kind="ExternalInput") for l in range(2)]
    d_whh = [nc.dram_tensor(f"whh{l}", [RH, 4 * RH], BF16, kind="ExternalInput")
             for l in range(2)]
    d_lb = [nc.dram_tensor(f"lb{l}", [P, NG], F32, kind="ExternalInput")
            for l in range(2)]
    d_rw1 = nc.dram_tensor("rw1", [RH, RHID], BF16, kind="ExternalInput")
    d_rw2 = nc.dram_tensor("rw2", [RHID, R_OUT], BF16, kind="ExternalInput")
    d_uw1 = nc.dram_tensor("uw1", [UIN, UH], BF16, kind="ExternalInput")
    d_uw2 = nc.dram_tensor("uw2", [UH, U_OUT], BF16, kind="ExternalInput")
    d_mlpb = nc.dram_tensor("mlpb", [P, 9], F32, kind="ExternalInput")
    d_expw = nc.dram_tensor("expw", [EPC, EIN, H2], BF16, kind="ExternalInput")
    d_w2r = nc.dram_tensor("w2r", [EPC // 2, P, EH], F32, kind="ExternalInput")
    d_b1p = nc.dram_tensor("b1p", [EPC, H2], BF16, kind="ExternalInput")
    d_b2bc = nc.dram_tensor("b2bc", [P, E], F32, kind="ExternalInput")
    d_out = nc.dram_tensor("out", [B, EPC], F32, kind="ExternalOutput")

    with tile.TileContext(nc) as tc:
        with (
            tc.tile_pool(name="sb", bufs=1) as sb,
            tc.tile_pool(name="dr", bufs=1, space="DRAM") as dr,
        ):
            # ---- latency-critical loads first (indices feed the gathers) ----
            ridx_t = sb.tile([BL, T], I32)
            nc.sync.dma_start(out=ridx_t[:], in_=d_ridx[:])
            uid_t = sb.tile([BL, 1], I32)
            nc.sync.dma_start(out=uid_t[:], in_=d_uid[:])
            Uin = sb.tile([P, UIN], F32)
            nc.gpsimd.indirect_dma_start(
                out=Uin[:, 0:UD], out_offset=None, in_=d_uemb[:],
                in_offset=bass.IndirectOffsetOnAxis(ap=uid_t[:, 0:1], axis=0))
            nc.sync.dma_start(out=Uin[:, UD:UIN], in_=d_udata[:])
            X = sb.tile([P, T, RD], BF16)
            for t in range(T):
                nc.gpsimd.indirect_dma_start(
                    out=X[:, t, :], out_offset=None, in_=d_remb[:],
                    in_offset=bass.IndirectOffsetOnAxis(ap=ridx_t[:, t:t + 1], axis=0))

            # ---- small static weights ----
            identb = sb.tile([P, P], BF16)
            make_identity(nc, identb[:])
            ident = sb.tile([P, P], F32)
            make_identity(nc, ident[:])
            wih_t = []
            whh_t = []
            for l in range(2):
                kin = RD if l == 0 else RH
                wt = []
                for kc in range(kin // P):
                    tl = sb.tile([P, 4 * RH], BF16, tag=f"wih{l}_{kc}")
                    nc.sync.dma_start(out=tl[:], in_=d_wih[l][kc * P:(kc + 1) * P, :])
                    wt.append(tl)
                wih_t.append(wt)
                ht = []
                for kc in range(2):
                    tl = sb.tile([P, 4 * RH], BF16, tag=f"whh{l}_{kc}")
                    nc.sync.dma_start(out=tl[:], in_=d_whh[l][kc * P:(kc + 1) * P, :])
                    ht.append(tl)
                whh_t.append(ht)
            lb_t = []
            for l in range(2):
                tl = sb.tile([P, NG], F32, tag=f"lb{l}")
                nc.sync.dma_start(out=tl[:], in_=d_lb[l][:])
                lb_t.append(tl)
            rw1_t = []
            for kc in range(2):
                tl = sb.tile([P, RHID], BF16, tag=f"rw1_{kc}")
                nc.sync.dma_start(out=tl[:], in_=d_rw1[kc * P:(kc + 1) * P, :])
                rw1_t.append(tl)
            rw2_t = []
            for kc in range(4):
                tl = sb.tile([P, R_OUT], BF16, tag=f"rw2_{kc}")
                nc.sync.dma_start(out=tl[:], in_=d_rw2[kc * P:(kc + 1) * P, :])
                rw2_t.append(tl)
            uw1_t = sb.tile([UIN, UH], BF16)
            nc.sync.dma_start(out=uw1_t[:], in_=d_uw1[:])
            uw2a = sb.tile([P, U_OUT], BF16)
            nc.sync.dma_start(out=uw2a[:], in_=d_uw2[0:P, :])
            uw2b = sb.tile([UH - P, U_OUT], BF16)
            nc.sync.dma_start(out=uw2b[:], in_=d_uw2[P:UH, :])
            mlpb_t = sb.tile([P, 9], F32)
            nc.sync.dma_start(out=mlpb_t[:], in_=d_mlpb[:])
            b2bc_t = sb.tile([P, E], F32)
            nc.sync.dma_start(out=b2bc_t[:], in_=d_b2bc[:])
            if bp["exp_b1"]:
                b1p_t = sb.tile([EPC, H2], BF16)
                nc.sync.dma_start(out=b1p_t[:], in_=d_b1p[:])
                ones1 = sb.tile([1, P], BF16)
                nc.gpsimd.memset(ones1[:], 1.0)
            # expert weights: big, needed only in phase 2 -> emitted last
            w1e_t = []
            for e in range(EPC):
                tl = sb.tile([P, EIN // P, H2], BF16, tag=f"w1e{e}")
                for i in range(EIN // P):
                    nc.sync.dma_start(out=tl[:, i, :],
                                      in_=d_expw[e, i * P:(i + 1) * P, :])
                w1e_t.append(tl)

            zT = sb.tile([P, EIN], BF16)

            with (
                tc.tile_pool(name="ptm", bufs=2, space="PSUM") as ptm,
                tc.tile_pool(name="pXG", bufs=1, space="PSUM") as pXG,
                tc.tile_pool(name="pG1", bufs=1, space="PSUM") as pG1,
            ):
                # ---- user MLP (independent of LSTM) ----
                tru = ptm.tile([P, P], F32, tag="tm")
                nc.tensor.transpose(out=tru[0:UIN, :], in_=Uin[:, :],
                                    identity=ident[:])
                UinT = sb.tile([UIN, P], BF16)
                nc.vector.tensor_copy(out=UinT[:], in_=tru[0:UIN, :])
                u1ps = ptm.tile([P, 2 * P], F32, tag="tm")
                nc.tensor.matmul(out=u1ps[:, 0:P], lhsT=uw1_t[:, 0:P],
                                 rhs=UinT[:], start=True, stop=True)
                nc.tensor.matmul(out=u1ps[0:UH - P, P:2 * P], lhsT=uw1_t[:, P:UH],
                                 rhs=UinT[:], start=True, stop=True)
                U1T = sb.tile([P, 2 * P], BF16)
                nc.scalar.activation(U1T[:, 0:P], u1ps[:, 0:P], AF.Relu,
                                     bias=mlpb_t[:, 6:7])
                nc.scalar.activation(U1T[0:UH - P, P:2 * P], u1ps[0:UH - P, P:2 * P],
                                     AF.Relu, bias=mlpb_t[0:UH - P, 7:8])
                u2ps = ptm.tile([P, P], F32, tag="tm")
                nc.tensor.matmul(out=u2ps[:], lhsT=uw2a[:], rhs=U1T[:, 0:P],
                                 start=True, stop=False)
                nc.tensor.matmul(out=u2ps[:], lhsT=uw2b[:], rhs=U1T[0:UH - P, P:2 * P],
                                 start=False, stop=True)
                nc.scalar.activation(zT[:, 0:P], u2ps[:], AF.Identity,
                                     bias=mlpb_t[:, 8:9])

                XT = sb.tile([P, T, RD], BF16)
                tr_insts = {}

                def transpose_x(t):
                    tr = ptm.tile([P, P], BF16, name="tr", tag="tm")
                    ti = nc.tensor.transpose(out=tr[:], in_=X[:, t, :],
                                             identity=identb[:])
                    nc.vector.tensor_copy(out=XT[:, t, :], in_=tr[:])
                    tr_insts[t] = ti

                for t in range(4):
                    transpose_x(t)

                # ---- LSTM scan ----
                XG = [pXG.tile([P, 4 * RH], F32, name="XG0"),
                      pXG.tile([P, 4 * RH], F32, name="XG1")]
                G1 = pG1.tile([P, 4 * RH], F32, name="G1")
                S = [sb.tile([P, 1280], F32, name=f"S{l}") for l in range(2)]
                TTs = [sb.tile([P, 512], F32, name=f"TT{l}") for l in range(2)]
                TC = [sb.tile([P, RH], F32, name=f"TC{l}") for l in range(2)]
                # h0 double-buffered: layer-1 (delayed one step) still needs
                # h0(t-1) after cell(0,t) has produced h0(t)
                hT0 = [sb.tile([P, RH], BF16, name=f"hT0_{j}") for j in range(2)]
                hT1 = sb.tile([P, RH], BF16, name="hT1")

                def cell(l, t, g, h):
                    # S layout: [sig_i | tanh_g | sig_f | sig_o | c], 256 each
                    s, tt_, tc = S[l], TTs[l], TC[l]
                    bias = (lambda jg: lb_t[l][:, jg:jg + 1]) if bp["lstm_bias"] \
                        else (lambda jg: 0.0)
                    if bp["lstm_bias"]:
                        for jg in (2, 3):
                            nc.scalar.activation(s[:, jg * P:(jg + 1) * P],
                                                 g[:, jg * P:(jg + 1) * P],
                                                 AF.Tanh, bias=bias(jg))
                        for jg in (0, 1):
                            nc.scalar.activation(s[:, jg * P:(jg + 1) * P],
                                                 g[:, jg * P:(jg + 1) * P],
                                                 AF.Sigmoid, bias=bias(jg))
                    else:
                        nc.scalar.activation(s[:, 256:512], g[:, 256:512], AF.Tanh)
                        nc.scalar.activation(s[:, 0:256], g[:, 0:256], AF.Sigmoid)
                    if t == 0:
                        nc.vector.tensor_tensor(out=s[:, 1024:1280], in0=s[:, 0:256],
                                                in1=s[:, 256:512], op=ALU.mult)
                    else:
                        # t1 = sig_i * tanh_g  (only needs bank0 of G)
                        nc.vector.tensor_tensor(out=tt_[:, 0:256], in0=s[:, 0:256],
                                                in1=s[:, 256:512], op=ALU.mult)
                    if bp["lstm_bias"]:
                        for jg in (4, 5, 6, 7):
                            nc.scalar.activation(s[:, jg * P:(jg + 1) * P],
                                                 g[:, jg * P:(jg + 1) * P],
                                                 AF.Sigmoid, bias=bias(jg))
                    else:
                        nc.scalar.activation(s[:, 512:1024], g[:, 512:1024],
                                             AF.Sigmoid)
                    if t > 0:
                        # t2 = sig_f * c ; c' = t1 + t2
                        nc.vector.tensor_tensor(out=tt_[:, 256:512],
                                                in0=s[:, 512:768],
                                                in1=s[:, 1024:1280], op=ALU.mult)
                        nc.vector.tensor_tensor(out=s[:, 1024:1280], in0=tt_[:, 0:256],
                                                in1=tt_[:, 256:512], op=ALU.add)
                    nc.scalar.activation(tc[:, :], s[:, 1024:1280], AF.Tanh)
                    nc.vector.tensor_tensor(out=h[:, :], in0=s[:, 768:1024],
                                            in1=tc[:, :], op=ALU.mult)

                # PSUM group discipline: start=True clears has_written for the
                # WHOLE bank (4 jg slices), so open each bank's group only on
                # its first slice and close it on the last.
                bank_first = lambda jg: jg % 4 == 0
                bank_last = lambda jg: jg % 4 == 3

                def l1_step(u):
                    """layer-1 MMs + cell for step u (issued one step late: at
                    issue time h0(u) is long ready -> no PE wait)."""
                    h0u = hT0[u % 2]
                    for jg in range(NG):
                        for kc in range(2):
                            nc.tensor.matmul(
                                out=G1[:, jg * P:(jg + 1) * P],
                                lhsT=wih_t[1][kc][:, jg * P:(jg + 1) * P],
                                rhs=h0u[:, kc * P:(kc + 1) * P],
                                start=(kc == 0 and bank_first(jg)),
                                stop=(u == 0 and kc == 1 and bank_last(jg)))
                    if u > 0:
                        for jg in range(NG):
                            for kc in range(2):
                                nc.tensor.matmul(
                                    out=G1[:, jg * P:(jg + 1) * P],
                                    lhsT=whh_t[1][kc][:, jg * P:(jg + 1) * P],
                                    rhs=hT1[:, kc * P:(kc + 1) * P],
                                    start=False, stop=(kc == 1 and bank_last(jg)))
                    cell(1, u, G1, hT1)

                # prologue: xg0 for t=0
                for jg in range(NG):
                    nc.tensor.matmul(out=XG[0][:, jg * P:(jg + 1) * P],
                                     lhsT=wih_t[0][0][:, jg * P:(jg + 1) * P],
                                     rhs=XT[:, 0, :], start=bank_first(jg),
                                     stop=bank_last(jg))

                for t in range(T):
                    Gx = XG[t % 2]
                    # L0 recurrent (the critical chain)
                    if t > 0:
                        for jg in range(NG):
                            for kc in range(2):
                                nc.tensor.matmul(
                                    out=Gx[:, jg * P:(jg + 1) * P],
                                    lhsT=whh_t[0][kc][:, jg * P:(jg + 1) * P],
                                    rhs=hT0[(t - 1) % 2][:, kc * P:(kc + 1) * P],
                                    start=False, stop=(kc == 1 and bank_last(jg)))
                    cell(0, t, Gx, hT0[t % 2])
                    # layer 1 for the previous step: all operands ready
                    if t > 0:
                        l1_step(t - 1)
                    # filler: xg0 for step t+1 (keeps PE warm, off-chain)
                    if t + 1 < T:
                        Gn = XG[(t + 1) % 2]
                        for jg in range(NG):
                            nc.tensor.matmul(
                                out=Gn[:, jg * P:(jg + 1) * P],
                                lhsT=wih_t[0][0][:, jg * P:(jg + 1) * P],
                                rhs=XT[:, t + 1, :], start=bank_first(jg), stop=False)
                    if t + 4 < T:
                        transpose_x(t + 4)
                l1_step(T - 1)

                # ---- ratings MLP ----
                r1ps = ptm.tile([P, RHID], F32, tag="tm")
                for mc in range(4):
                    for kc in range(2):
                        nc.tensor.matmul(
                            out=r1ps[:, mc * P:(mc + 1) * P],
                            lhsT=rw1_t[kc][:, mc * P:(mc + 1) * P],
                            rhs=hT1[:, kc * P:(kc + 1) * P],
                            start=(kc == 0), stop=(kc == 1))
                R1T = sb.tile([P, RHID], BF16)
                for mc in range(4):
                    nc.scalar.activation(R1T[:, mc * P:(mc + 1) * P],
                                         r1ps[:, mc * P:(mc + 1) * P], AF.Relu,
                                         bias=mlpb_t[:, mc:mc + 1])
                r2ps = ptm.tile([P, R_OUT], F32, tag="tm")
                for mc in range(2):
                    for kc in range(4):
                        nc.tensor.matmul(
                            out=r2ps[:, mc * P:(mc + 1) * P],
                            lhsT=rw2_t[kc][:, mc * P:(mc + 1) * P],
                            rhs=R1T[:, kc * P:(kc + 1) * P],
                            start=(kc == 0), stop=(kc == 3))
                for mc in range(2):
                    nc.scalar.activation(zT[:, P + mc * P:P + (mc + 1) * P],
                                         r2ps[:, mc * P:(mc + 1) * P], AF.Identity,
                                         bias=mlpb_t[:, 4 + mc:5 + mc])

            # ---- allgather z ----
            zdr = dr.tile([P, EIN], BF16)
            nc.sync.dma_start(out=zdr[:], in_=zT[:])
            Zall = dr.tile([NCORES, P, EIN], BF16, addr_space="Shared")
            if sim_single:
                nc.sync.dma_start(out=Zall[0], in_=zdr[:])
            else:
                nc.gpsimd.collective_compute(
                    "AllGather", ALU.bypass, ins=[zdr.opt()], outs=[Zall.opt()],
                    replica_groups=[list(range(NCORES))])

            # ---- experts ----
            with tc.tile_pool(name="phe", bufs=2, space="PSUM") as phe:
                crange = [0] if sim_single else list(range(NCORES))
                Zt = []
                for c in crange:
                    tl = sb.tile([P, EIN], BF16, tag=f"zt{c}")
                    nc.sync.dma_start(out=tl[:], in_=Zall[c])
                    Zt.append(tl)
                scr = sb.tile([P, H2], BF16)
                scrd = sb.tile([P, EH], BF16)
                splus = sb.tile([P, E], F32)
                sminus = sb.tile([P, E], F32)
                souts = sb.tile([P, E], F32)
                outs = sb.tile([P, E], F32)
                with tc.tile_pool(name="sw2", bufs=2) as sw2:
                    pair_order = []
                    for eh in range(EPC // 2):
                        for c in crange:
                            pair_order.append((2 * eh, c))
                            pair_order.append((2 * eh + 1, c))
                    w2ts = {}
                    for e, c in pair_order:
                        act_path = e % 2 == 0
                        chunks = NH_full if act_path else NH_nat
                        if not act_path and e not in w2ts:
                            w2t = sw2.tile([P, EH], F32, name="w2t", tag="w2t")
                            nc.sync.dma_start(out=w2t[:], in_=d_w2r[e // 2])
                            w2ts = {e: w2t}
                        if True:
                            ci = crange.index(c)
                            he = phe.tile([P, H2], F32, name="he", tag="he")
                            for i in range(EIN // P):
                                first = i == 0
                                last = (i == EIN // P - 1) and not bp["exp_b1"]
                                for (n0, n1) in chunks:
                                    nc.tensor.matmul(
                                        out=he[:, n0:n1],
                                        lhsT=Zt[ci][:, i * P:(i + 1) * P],
                                        rhs=w1e_t[e][:, i, n0:n1],
                                        start=first, stop=last)
                            if bp["exp_b1"]:
                                for (n0, n1) in chunks:
                                    nc.tensor.matmul(
                                        out=he[:, n0:n1], lhsT=ones1[:],
                                        rhs=b1p_t[e:e + 1, n0:n1],
                                        start=False, stop=True)
                            col = c * EPC + e
                            if act_path:
                                nc.scalar.activation(
                                    scr[:, 0:Bpos], he[:, 0:Bpos], AF.Relu,
                                    accum_out=splus[:, col:col + 1])
                                nc.scalar.activation(
                                    scr[:, Bpos:H2], he[:, Bpos:H2], AF.Relu,
                                    accum_out=sminus[:, col:col + 1])
                            else:
                                nc.vector.scalar_tensor_tensor(
                                    out=scrd[:], in0=he[:, 0:EH], scalar=0.0,
                                    in1=w2t[:], op0=ALU.max, op1=ALU.mult,
                                    accum_out=souts[:, col:col + 1])
                n_used = len(crange) * EPC
                # even-slot columns: souts = splus - sminus (strided over e)
                for c in crange:
                    base = c * EPC
                    nc.vector.tensor_tensor(
                        out=souts[:, base:base + EPC:2],
                        in0=splus[:, base:base + EPC:2],
                        in1=sminus[:, base:base + EPC:2], op=ALU.subtract)
                if bp["exp_b2"]:
                    nc.vector.tensor_tensor(out=outs[:, 0:n_used],
                                            in0=souts[:, 0:n_used],
                                            in1=b2bc_t[:, 0:n_used], op=ALU.add)
                    fin = outs
                else:
                    fin = souts
                for c in crange:
                    nc.sync.dma_start(out=d_out[c * P:(c + 1) * P, :],
                                      in_=fin[:, c * EPC:(c + 1) * EPC])
    nc.finalize()
    return nc


def _get_nc(bp, sim_single=False):
    key = (bp["lstm_bias"], bp["exp_b1"], bp["exp_b2"], sim_single)
    if key not in _cache:
        _cache[key] = _build(bp, sim_single=sim_single)
    return _cache[key]


def run(inputs, trace=False):
    in_maps, bp = _prep(inputs)
    nc = _get_nc(bp)
    res = run_bass_kernel_spmd(nc, in_maps, core_ids=list(range(NCORES)),
                               trace=trace)
    out = np.concatenate([np.asarray(res.results[c]["out"]) for c in range(NCORES)],
                         axis=1).astype(np.float32)
    return out, res


def kernel(**inputs) -> np.ndarray:
    out, _ = run(inputs, trace=False)
    return out

